# revision 11
# baseline (speedup 1.0000x reference)
"""Trainium2 Bass kernel for a dense transformer block (pre-LN, 16-head causal
attention + 3x FFN), distributed over 8 NeuronCores.

Sharding: tensor-parallel over heads (2 heads/core, both batch elements on
every core) for LN1/QKV/attention; one 8-core AllToAll redistributes the
per-head attention context to token-parallel shards (512 tokens/core) for the
output projection, LN2 and the FFN.  Matmuls run in bf16 with f32 PSUM
accumulation; the residual stream stays f32.

All layouts are transposed ([channel, token]) on chip so every matmul
contracts over the partition dim.  LayerNorm 1 is folded into the QKV weights
(rank-2 correction matmul per accumulation group); LayerNorm 2 is folded into
the first FFN matmul (rank-1 -mu*colsum(W1) correction + per-token 1/std
scale at eviction).

Scheduling notes:
 - the LN1 stats path (bn_stats -> tiny AllGather) is emitted at high
   priority and the bulk x^T/weight DMAs are pushed past it with
   tile_wait_until, so the collective's SDMA traffic isn't stuck behind
   megabytes of queued descriptors
 - attention interleaves the two batches' q-chunks so one chunk's softmax
   drain hides under the other's score/AV stream; softmax normalization is
   deferred into the next round's PE stream
 - every DMA is layed out >=2KB-contiguous per partition on both sides
"""

import numpy as np
import ml_dtypes

B, T, C = 2, 2048, 1024
NH, H = 16, 64
FF = 3 * C
EPS = 1e-6
N_CORES = 8
TT = B * T            # 4096 tokens processed per core (head-parallel phase)
TS = TT // N_CORES    # 512 tokens per core (token-parallel phase)
HPC = NH // N_CORES   # 2 heads per core
HD2 = HPC * H         # 128

BF16 = ml_dtypes.bfloat16

_BUILT = {}

NT = TT // 128        # 32 token tiles
NKC = C // 128        # 8 channel k-tiles
NMF = FF // 128       # 24 ff tiles
NCH = TT // 512       # 8 512-token chunks


def _build():
    import concourse.bacc as bacc
    import concourse.mybir as mybir
    import concourse.tile as tile
    dt = mybir.dt
    alu = mybir.AluOpType
    act = mybir.ActivationFunctionType

    nc = bacc.Bacc("TRN2", target_bir_lowering=False, debug=False,
                   num_devices=N_CORES)

    # ----- kernel I/O (per-core shards) -----
    # layouts chosen so every DMA is contiguous per SBUF partition
    p_x = nc.declare_dram_parameter("p_x", [TT // N_CORES, C], dt.bfloat16, isOutput=False)
    p_xT = nc.declare_dram_parameter("p_xT", [NCH, 128, NKC, 512], dt.bfloat16, isOutput=False)
    p_xTs = nc.declare_dram_parameter("p_xTs", [128, NKC, TS], dt.float32, isOutput=False)
    p_wq = nc.declare_dram_parameter("p_wq", [128, NKC, HD2], dt.bfloat16, isOutput=False)
    p_wk = nc.declare_dram_parameter("p_wk", [128, NKC, HD2], dt.bfloat16, isOutput=False)
    p_wv = nc.declare_dram_parameter("p_wv", [128, NKC, HD2], dt.bfloat16, isOutput=False)
    p_cq = nc.declare_dram_parameter("p_cq", [2, HD2], dt.bfloat16, isOutput=False)
    p_ck = nc.declare_dram_parameter("p_ck", [2, HD2], dt.bfloat16, isOutput=False)
    p_cv = nc.declare_dram_parameter("p_cv", [2, HD2], dt.bfloat16, isOutput=False)
    p_woblk = nc.declare_dram_parameter("p_woblk", [NKC, 128, NKC, 128], dt.bfloat16, isOutput=False)
    p_bo = nc.declare_dram_parameter("p_bo", [1, C], dt.bfloat16, isOutput=False)
    p_w1blk = nc.declare_dram_parameter("p_w1blk", [NMF, 128, NKC, 128], dt.bfloat16, isOutput=False)
    p_b1c = nc.declare_dram_parameter("p_b1c", [128, NMF], dt.float32, isOutput=False)
    p_cs1 = nc.declare_dram_parameter("p_cs1", [1, FF], dt.bfloat16, isOutput=False)
    p_w2blk = nc.declare_dram_parameter("p_w2blk", [NKC, 128, NMF, 128], dt.bfloat16, isOutput=False)
    p_b2 = nc.declare_dram_parameter("p_b2", [1, C], dt.bfloat16, isOutput=False)
    p_maskd = nc.declare_dram_parameter("p_maskd", [128, 128], dt.bfloat16, isOutput=False)
    p_ident = nc.declare_dram_parameter("p_ident", [128, 128], dt.bfloat16, isOutput=False)
    p_out = nc.declare_dram_parameter("p_out", [C, TS], dt.float32, isOutput=True)

    with tile.TileContext(nc, num_cores=N_CORES) as tc:
        with (
            tc.tile_pool(name="persist", bufs=1) as pp,
            tc.tile_pool(name="dram", bufs=1, space="DRAM") as pdram,
        ):
            # ------------- persistent constants & activation tensors -------------
            rows_all = pp.tile([2, TT], dt.bfloat16)
            inv_row = pp.tile([1, TT], dt.bfloat16)
            inv_b = pp.tile([128, TT], dt.bfloat16)
            qT = pp.tile([128, TT], dt.bfloat16)
            kT = pp.tile([128, TT], dt.bfloat16)
            v = pp.tile([128, NT, 2, 65], dt.bfloat16)
            ctxT = pp.tile([128, TT], dt.bfloat16)
            ident = pp.tile([128, 128], dt.bfloat16)
            maskd = pp.tile([128, 128], dt.bfloat16)
            ones_row = pp.tile([1, 512], dt.bfloat16)
            ones128_row = pp.tile([1, 128], dt.bfloat16)
            isc_col = pp.tile([128, 1], dt.bfloat16)   # 1/1024 column for LN2 sums
            cq = pp.tile([2, HD2], dt.bfloat16)
            ck = pp.tile([2, HD2], dt.bfloat16)
            cv = pp.tile([2, HD2], dt.bfloat16)
            wq = pp.tile([128, NKC, HD2], dt.bfloat16)
            wk = pp.tile([128, NKC, HD2], dt.bfloat16)
            wv = pp.tile([128, NKC, HD2], dt.bfloat16)

            cc_in = pdram.tile([N_CORES, 128, TS], dt.bfloat16)
            cc_out = pdram.tile([N_CORES, 128, TS], dt.bfloat16)

            # ---------------- stage A: LN1 stats (sharded) + QKV ----------------
            with (
                tc.tile_pool(name="xtpool", bufs=1) as pxt,
                tc.tile_pool(name="stat", bufs=1) as pst,
                tc.tile_pool(name="apsum", bufs=3, space="PSUM") as pps_a,
                tc.tile_pool(name="apsum1", bufs=1, space="PSUM") as pps_a1,
                tc.tile_pool(name="vtpsum", bufs=2, space="PSUM") as pps_vt,
            ):
                NLT = NT // N_CORES        # 4 local token tiles
                # ---- stats path first, at high priority: nothing bulky may
                # delay the tiny AllGather ----
                with tc.high_priority():
                    nc.sync.dma_start(ident[:], p_ident[:])
                    xla = pst.tile([128, NLT, C], dt.bfloat16)
                    nc.sync.dma_start(
                        xla[:], p_x.ap().rearrange("(i p) c -> p i c", p=128))
                    stats = pst.tile([128, NLT, 2], dt.float32)
                    for i in range(NLT):
                        bnt = pst.tile([128, 2, 6], dt.float32, tag="bnt")
                        nc.vector.bn_stats(bnt[:, 0, :], xla[:, i, 0:512])
                        nc.vector.bn_stats(bnt[:, 1, :], xla[:, i, 512:1024])
                        nc.vector.bn_aggr(stats[:, i, :], bnt[:])

                    # (negmu, std+eps, inv) for the local 512 tokens
                    stat2 = pst.tile([128, NLT, 2], dt.bfloat16)
                    stdf = pst.tile([128, NLT], dt.float32)
                    nc.scalar.activation(stdf[:], stats[:, :, 1], act.Sqrt,
                                         scale=float(C) / (C - 1))
                    nc.vector.tensor_scalar(stdf[:], stdf[:], EPS, None, alu.add)
                    invf = pst.tile([128, NLT], dt.float32)
                    nc.vector.reciprocal(invf[:], stdf[:])
                    nc.vector.tensor_scalar(stat2[:, :, 0], stats[:, :, 0], -1.0,
                                            None, alu.mult)
                    nc.vector.tensor_copy(stat2[:, :, 1], stdf[:])
                    statinv = pst.tile([128, NLT], dt.bfloat16)
                    nc.vector.tensor_copy(statinv[:], invf[:])

                    rows_loc = pst.tile([2, TS], dt.bfloat16)
                    rows_locv = pst.tile([1, TS], dt.bfloat16)
                    for i in range(NLT):
                        pt = pps_a1.tile([2, 128], dt.bfloat16, tag="rowtp")
                        nc.tensor.transpose(pt[:], stat2[:, i, :], ident[:])
                        nc.vector.tensor_copy(rows_loc[:, 128 * i:128 * (i + 1)], pt[:])
                        ptv = pps_a1.tile([1, 128], dt.bfloat16, tag="rowtpv")
                        nc.tensor.transpose(ptv[:], statinv[:, i:i + 1], ident[:])
                        nc.vector.tensor_copy(rows_locv[:, 128 * i:128 * (i + 1)], ptv[:])

                    # all-gather the stat rows (tiny, latency-bound)
                    st_in = pdram.tile([3, TS], dt.bfloat16)
                    st_out = pdram.tile([N_CORES, 3, TS], dt.bfloat16)
                    nc.sync.dma_start(st_in[0:2, :], rows_loc[:])
                    nc.sync.dma_start(st_in[2:3, :], rows_locv[:])
                    nc.gpsimd.collective_compute(
                        "AllGather", alu.bypass,
                        replica_groups=[list(range(N_CORES))],
                        ins=[st_in.opt()],
                        outs=[st_out.opt()],
                    )
                    nc.sync.dma_start(
                        rows_all[:].rearrange("s (r t) -> s r t", r=N_CORES),
                        st_out[:, 0:2, :].rearrange("r s t -> s r t"))
                    nc.sync.dma_start(
                        inv_row[:].rearrange("s (r t) -> s r t", r=N_CORES),
                        st_out[:, 2:3, :].rearrange("r s t -> s r t"))

                # ---- weights (normal priority) ----
                nc.sync.dma_start(wq[:], p_wq[:])
                nc.sync.dma_start(wk[:], p_wk[:])
                nc.sync.dma_start(wv[:], p_wv[:])
                nc.sync.dma_start(cq[:], p_cq[:])
                nc.sync.dma_start(ck[:], p_ck[:])
                nc.sync.dma_start(cv[:], p_cv[:])
                nc.sync.dma_start(maskd[:], p_maskd[:])
                nc.vector.memset(ones_row[:], 1.0)
                nc.vector.memset(ones128_row[:], 1.0)
                nc.vector.memset(isc_col[:], 1.0 / C)

                # x^T chunk-major so each chunk DMA is one 8KB/partition blob;
                # held back so the stats collective owns the wire first
                xT = pxt.tile([128, NCH, NKC, 512], dt.bfloat16)
                for ch in range(NCH):
                    with tc.tile_wait_until(0.012 + 0.002 * ch):
                        nc.sync.dma_start(xT[:, ch], p_xT[ch])

                vT = pxt.tile([128, TT], dt.bfloat16)
                nc.vector.memset(v[:, :, :, 64], 1.0)
                for ch in range(NCH):
                    sl = slice(512 * ch, 512 * (ch + 1))
                    for (nm, w, cw, dst) in (("q", wq, cq, qT), ("k", wk, ck, kT),
                                             ("v", wv, cv, vT)):
                        ps = pps_a.tile([128, 512], dt.float32,
                                        name=f"ps{nm}", tag="qkv")
                        for k in range(NKC):
                            nc.tensor.matmul(ps[:], w[:, k, :], xT[:, ch, k, :],
                                             start=(k == 0), stop=False)
                        if nm == "q":
                            # inv broadcast for this chunk: after the q mains
                            # (PE runway for the stats collective), before any
                            # eviction reads it
                            pb = pps_a1.tile([128, 512], dt.float32, tag="invb")
                            nc.tensor.matmul(pb[:], ones128_row[:],
                                             inv_row[0:1, sl],
                                             start=True, stop=True)
                            nc.scalar.copy(inv_b[:, sl], pb[:])
                        nc.tensor.matmul(ps[:], cw[:], rows_all[0:2, sl],
                                         start=False, stop=True)
                        nc.vector.tensor_tensor(dst[:, sl], ps[:], inv_b[:, sl],
                                                alu.mult)
                    # v_aug [s, tile, head, 65] via PE transpose, interleaved
                    # with the QKV matmul stream to keep the PE warm
                    for i in range(4 * ch, 4 * ch + 4):
                        pvt = pps_vt.tile([128, 128], dt.bfloat16, tag="vtp")
                        nc.tensor.transpose(pvt[:], vT[:, 128 * i:128 * (i + 1)],
                                            ident[:])
                        if i % 2 == 0:
                            nc.scalar.copy(v[:, i, :, 0:64],
                                           pvt[:].rearrange("p (h d) -> p h d", h=2))
                        else:
                            nc.vector.tensor_copy(v[:, i, :, 0:64],
                                                  pvt[:].rearrange("p (h d) -> p h d", h=2))

            # ---------------- stage B: attention ----------------
            # two batches interleaved: batch b0's chunk and b1's chunk of the
            # same size run j-step-by-j-step so each hides the other's
            # softmax/normalize bubbles.
            with (
                tc.tile_pool(name="exps", bufs=4) as pexp,
                tc.tile_pool(name="attsb", bufs=2) as pat,
                tc.tile_pool(name="scpsum", bufs=4, space="PSUM") as pps_sc,
                tc.tile_pool(name="ctxpsum", bufs=1, space="PSUM") as pps_ctx,
            ):
                def emit_norm(stt):
                    pcs_, zinvb_, gsl_, cidx_ = stt
                    pzb = pps_sc.tile([128, 512], dt.float32, name="pzb", tag="sc")
                    for h in range(2):
                        nc.tensor.matmul(pzb[64 * h:64 * (h + 1), :],
                                         ones128_row[0:1, 0:64], zinvb_[h][:],
                                         start=True, stop=True)
                    zb = pat.tile([128, 512], dt.bfloat16, tag="zbs")
                    nc.vector.tensor_copy(zb[:], pzb[:])
                    for h in range(2):
                        nc.vector.tensor_tensor(
                            ctxT[64 * h:64 * (h + 1), gsl_],
                            pcs_[h][0:64, :], zb[64 * h:64 * (h + 1), :],
                            alu.mult)
                    nc.sync.dma_start(cc_in[cidx_], ctxT[:, gsl_])

                pending = []
                for qt in range(T // 512):
                    nj = 4 * qt + 4
                    gsl = {b: slice(b * T + 512 * qt, b * T + 512 * qt + 512)
                           for b in range(B)}
                    pc = None
                    ets = {b: [] for b in range(B)}
                    for j in range(nj):
                        for b in range(B):
                            st = b * (T // 128) + j   # global s-tile index
                            et2 = []
                            for h in range(2):
                                hsl = slice(64 * h, 64 * (h + 1))
                                ps = pps_sc.tile([128, 512], dt.float32,
                                                 name="ps", tag="sc")
                                nc.tensor.matmul(
                                    ps[:], kT[hsl, 128 * st:128 * (st + 1)],
                                    qT[hsl, gsl[b]], start=True, stop=True)
                                et = pexp.tile([128, 512], dt.bfloat16,
                                               name=f"et{b}{h}", tag=f"et{b}{h}")
                                if j >= nj - 4:
                                    off = j - (nj - 4)
                                    if off > 0:
                                        nc.gpsimd.memset(et[:, 0:128 * off], 0.0)
                                    nc.scalar.activation(
                                        et[:, 128 * off:512], ps[:, 128 * off:512],
                                        act.Exp, scale=1.0 / float(np.sqrt(H)))
                                    nc.gpsimd.tensor_tensor(
                                        et[:, 128 * off:128 * (off + 1)],
                                        et[:, 128 * off:128 * (off + 1)],
                                        maskd[:], alu.mult)
                                else:
                                    nc.scalar.activation(et[:], ps[:], act.Exp,
                                                         scale=1.0 / float(np.sqrt(H)))
                                et2.append(et)
                            ets[b].append(et2)
                        if j == 0 and pending:
                            for stt in pending:
                                emit_norm(stt)
                            pending = []
                        if j == 1:
                            pc = {b: [pps_ctx.tile([65, 512], dt.float32,
                                                   name=f"pc{b}{h}", tag=f"ctx{b}{h}")
                                      for h in range(2)] for b in range(B)}
                        if j > 0:
                            for b in range(B):
                                for h in range(2):
                                    nc.tensor.matmul(
                                        pc[b][h][:], v[:, b * (T // 128) + j - 1, h, :],
                                        ets[b][j - 1][h][:],
                                        start=(j - 1 == 0), stop=False)
                    for b in range(B):
                        for h in range(2):
                            nc.tensor.matmul(
                                pc[b][h][:], v[:, b * (T // 128) + nj - 1, h, :],
                                ets[b][nj - 1][h][:],
                                start=(nj == 1), stop=True)
                    # 1/Z (row 64 of each ctx psum) on the vector engine
                    for b in range(B):
                        zinvb = []
                        for h in range(2):
                            zrow = pat.tile([1, 512], dt.float32,
                                            name=f"zrow{b}{h}", tag=f"z{b}{h}")
                            nc.vector.tensor_copy(zrow[:], pc[b][h][64:65, :])
                            zinv = pat.tile([1, 512], dt.float32,
                                            name=f"zinv{b}{h}", tag=f"zi{b}{h}")
                            nc.vector.reciprocal_approx_fast(zinv[:], zrow[:])
                            zib = pat.tile([1, 512], dt.bfloat16,
                                           name=f"zinvb{b}{h}", tag=f"zib{b}{h}")
                            nc.vector.tensor_copy(zib[:], zinv[:])
                            zinvb.append(zib)
                        pending.append((pc[b], zinvb, gsl[b],
                                        b * (T // 512) + qt))
                for stt in pending:
                    emit_norm(stt)
                pending = []

            # ---------------- AllToAll: heads -> tokens ----------------
            nc.gpsimd.collective_compute(
                "AllToAll", alu.bypass,
                replica_groups=[list(range(N_CORES))],
                ins=[cc_in.opt()],
                outs=[cc_out.opt()],
            )

            # ---------------- stage C: Wo + LN2 + FFN ----------------
            with (
                tc.tile_pool(name="postsb", bufs=1) as pq,
                tc.tile_pool(name="wstream", bufs=3) as pw,
                tc.tile_pool(name="evict", bufs=3) as pev,
                tc.tile_pool(name="ln2tmp", bufs=1) as pl2,
                tc.tile_pool(name="ffpsum", bufs=2, space="PSUM") as pps_ff,
                tc.tile_pool(name="cpsum", bufs=1, space="PSUM") as pps_c,
            ):
                ctxF = pq.tile([128, NKC, TS], dt.bfloat16)
                nc.sync.dma_start(ctxF[:],
                                  cc_out[:].rearrange("c p t -> p c t"))

                bo = pq.tile([1, C], dt.bfloat16)
                b2 = pq.tile([1, C], dt.bfloat16)
                b1c = pq.tile([128, NMF], dt.float32)
                cs1 = pq.tile([1, FF], dt.bfloat16)
                xTs = pq.tile([128, NKC, TS], dt.float32)
                with tc.tile_wait_until(0.12):
                    nc.sync.dma_start(bo[:], p_bo[:])
                    nc.sync.dma_start(b2[:], p_b2[:])
                    nc.sync.dma_start(b1c[:], p_b1c[:])
                    nc.sync.dma_start(cs1[:], p_cs1[:])
                    nc.sync.dma_start(xTs[:], p_xTs[:])

                # Wo + residual; LN2 column sums interleaved per block
                r2T = pq.tile([128, NKC, TS], dt.float32)
                r2b = pl2.tile([128, NKC, TS], dt.bfloat16)
                sq = pl2.tile([128, NKC, TS], dt.bfloat16)
                ps1 = pps_c.tile([1, TS], dt.float32, tag="s1")
                ps2 = pps_c.tile([1, TS], dt.float32, tag="s2")
                for mc in range(NKC):
                    wo_blk = pw.tile([128, NKC, 128], dt.bfloat16, tag="wo")
                    with tc.tile_wait_until(0.12):
                        nc.sync.dma_start(wo_blk[:], p_woblk[mc])
                    ps = pps_ff.tile([128, TS], dt.float32, tag="ff")
                    for k in range(NKC):
                        nc.tensor.matmul(ps[:], wo_blk[:, k, :], ctxF[:, k, :],
                                         start=(k == 0), stop=False)
                    nc.tensor.matmul(ps[:], bo[0:1, 128 * mc:128 * (mc + 1)],
                                     ones_row[:], start=False, stop=True)
                    nc.vector.tensor_tensor(r2T[:, mc, :], ps[:], xTs[:, mc, :],
                                            alu.add)
                    nc.scalar.copy(r2b[:, mc, :], r2T[:, mc, :])
                    nc.vector.tensor_tensor(sq[:, mc, :], r2b[:, mc, :],
                                            r2b[:, mc, :], alu.mult)
                    nc.tensor.matmul(ps1[:], isc_col[:], r2b[:, mc, :],
                                     start=(mc == 0), stop=(mc == NKC - 1))
                    nc.tensor.matmul(ps2[:], isc_col[:], sq[:, mc, :],
                                     start=(mc == 0), stop=(mc == NKC - 1))

                # ---- LN2 scalars: -mu row and broadcast 1/(std+eps) ----
                muf = pl2.tile([1, TS], dt.float32)
                nc.vector.tensor_copy(muf[:], ps1[:])
                negmu2 = pl2.tile([1, TS], dt.bfloat16)
                nc.vector.tensor_scalar(negmu2[:], muf[:], -1.0, None, alu.mult)
                varf = pl2.tile([1, TS], dt.float32)
                nc.vector.tensor_tensor(varf[:], muf[:], muf[:], alu.mult)
                nc.vector.tensor_tensor(varf[:], ps2[:], varf[:], alu.subtract)
                stdf2 = pl2.tile([1, TS], dt.float32)
                nc.scalar.activation(stdf2[:], varf[:], act.Sqrt,
                                     scale=float(C) / (C - 1))
                nc.vector.tensor_scalar(stdf2[:], stdf2[:], EPS, None, alu.add)
                inv2 = pl2.tile([1, TS], dt.float32)
                nc.vector.reciprocal_approx_fast(inv2[:], stdf2[:])
                inv2row = pl2.tile([1, TS], dt.bfloat16)
                nc.vector.tensor_copy(inv2row[:], inv2[:])
                pib = pps_c.tile([128, TS], dt.float32, tag="bcast")
                nc.tensor.matmul(pib[:], ones128_row[:], inv2row[:],
                                 start=True, stop=True)
                i2b = pl2.tile([128, TS], dt.bfloat16)
                nc.scalar.copy(i2b[:], pib[:])

                # ---- FFN (LN2 folded into the W1 matmul) ----
                hT = pq.tile([128, NMF, TS], dt.bfloat16)
                for mf in range(NMF):
                    w1_blk = pw.tile([128, NKC, 128], dt.bfloat16, tag="w1")
                    with tc.tile_wait_until(0.15):
                        nc.sync.dma_start(w1_blk[:], p_w1blk[mf])
                    ps = pps_ff.tile([128, TS], dt.float32, tag="ff")
                    for k in range(NKC):
                        nc.tensor.matmul(ps[:], w1_blk[:, k, :], r2b[:, k, :],
                                         start=(k == 0), stop=False)
                    nc.tensor.matmul(ps[:], cs1[0:1, 128 * mf:128 * (mf + 1)],
                                     negmu2[:], start=False, stop=True)
                    tmp = pev.tile([128, TS], dt.bfloat16, tag="htmp")
                    nc.vector.tensor_tensor(tmp[:], ps[:], i2b[:], alu.mult)
                    nc.vector.tensor_scalar(hT[:, mf, :], tmp[:], b1c[:, mf:mf + 1],
                                            0.0, alu.add, alu.max)

                for mc in range(NKC):
                    w2_blk = pw.tile([128, NMF, 128], dt.bfloat16, tag="w2")
                    with tc.tile_wait_until(0.18):
                        nc.sync.dma_start(w2_blk[:], p_w2blk[mc])
                    ps = pps_ff.tile([128, TS], dt.float32, tag="ff")
                    for k in range(NMF):
                        nc.tensor.matmul(ps[:], w2_blk[:, k, :], hT[:, k, :],
                                         start=(k == 0), stop=False)
                    nc.tensor.matmul(ps[:], b2[0:1, 128 * mc:128 * (mc + 1)],
                                     ones_row[:], start=False, stop=True)
                    ot = pev.tile([128, TS], dt.float32, tag="ot")
                    nc.vector.tensor_tensor(ot[:], ps[:], r2T[:, mc, :], alu.add)
                    nc.sync.dma_start(p_out[128 * mc:128 * (mc + 1), :], ot[:])

    nc.compile()
    return nc


def _host_prep(inputs):
    """Fold layernorm affine params into weights; build per-core input maps."""
    x = np.asarray(inputs["x"], np.float32)
    Wq = np.asarray(inputs["Wq"], np.float32)
    Wk = np.asarray(inputs["Wk"], np.float32)
    Wv = np.asarray(inputs["Wv"], np.float32)
    Wo = np.asarray(inputs["Wo"], np.float32)
    bo = np.asarray(inputs["bo"], np.float32)
    W1 = np.asarray(inputs["W1"], np.float32)
    b1 = np.asarray(inputs["b1"], np.float32)
    W2 = np.asarray(inputs["W2"], np.float32)
    b2 = np.asarray(inputs["b2"], np.float32)
    g1 = np.asarray(inputs["g1"], np.float32)
    be1 = np.asarray(inputs["be1"], np.float32)
    g2 = np.asarray(inputs["g2"], np.float32)
    be2 = np.asarray(inputs["be2"], np.float32)

    xf = x.reshape(TT, C)                      # both batches stacked
    xT = np.ascontiguousarray(xf.T)            # [C, TT]
    # chunk-major, partition-contiguous: [NCH, 128, NKC, 512]
    xT_blk = np.ascontiguousarray(
        xT.reshape(NKC, 128, NCH, 512).transpose(2, 1, 0, 3))

    def fold_qkv(W):
        Weff = g1[:, None] * W                  # [NH, C, H] with g1 on C
        Weff = np.ascontiguousarray(np.transpose(Weff, (1, 0, 2)))  # [C, NH, H]
        bias = np.einsum("c,hck->hk", be1, W)   # [NH, H]
        colsum = Weff.sum(axis=0)               # [NH, H]
        return Weff, bias, colsum

    Wq_e, bq, csq = fold_qkv(Wq)
    Wk_e, bk, csk = fold_qkv(Wk)
    Wv_e, bv, csv = fold_qkv(Wv)

    woT = np.ascontiguousarray(Wo.T)            # [NH*H, C]
    w1T = np.ascontiguousarray(g2[:, None] * W1.T)   # [C, FF]
    b1_eff = b1 + be2 @ W1.T                         # [FF]
    cs1 = w1T.sum(axis=0)                            # [FF]
    w2T = np.ascontiguousarray(W2.T)            # [FF, C]

    # blocked weights [mblk, 128, K/128, 128], contiguous per partition
    woblk = np.ascontiguousarray(
        woT.reshape(NKC, 128, NKC, 128).transpose(2, 1, 0, 3))
    w1blk = np.ascontiguousarray(
        w1T.reshape(NKC, 128, NMF, 128).transpose(2, 1, 0, 3))
    w2blk = np.ascontiguousarray(
        w2T.reshape(NMF, 128, NKC, 128).transpose(2, 1, 0, 3))

    tq = np.arange(128)[None, :]
    s = np.arange(128)[:, None]
    maskd = (s <= tq).astype(BF16)

    x_bf = xf.astype(BF16)
    shared = {
        "p_xT": xT_blk.astype(BF16),
        "p_woblk": woblk.astype(BF16),
        "p_bo": bo[None, :].astype(BF16),
        "p_w1blk": w1blk.astype(BF16),
        "p_b1c": np.ascontiguousarray(
            b1_eff.reshape(NMF, 128).T).astype(np.float32),
        "p_cs1": cs1[None, :].astype(BF16),
        "p_w2blk": w2blk.astype(BF16),
        "p_b2": b2[None, :].astype(BF16),
        "p_maskd": maskd,
        "p_ident": np.eye(128, dtype=np.float32).astype(BF16),
    }

    in_maps = []
    for r in range(N_CORES):
        h0 = HPC * r
        hs = slice(h0, h0 + HPC)
        b_r, s_r = divmod(r, N_CORES // B)
        tok = slice(s_r * TS, (s_r + 1) * TS)
        xTs = np.ascontiguousarray(
            x[b_r].T[:, tok].reshape(NKC, 128, TS).transpose(1, 0, 2))
        m = dict(shared)
        m["p_x"] = x_bf[r * (TT // N_CORES):(r + 1) * (TT // N_CORES), :]
        m["p_xTs"] = xTs.astype(np.float32)
        m["p_wq"] = np.ascontiguousarray(
            Wq_e[:, hs, :].reshape(NKC, 128, HD2).transpose(1, 0, 2)).astype(BF16)
        m["p_wk"] = np.ascontiguousarray(
            Wk_e[:, hs, :].reshape(NKC, 128, HD2).transpose(1, 0, 2)).astype(BF16)
        m["p_wv"] = np.ascontiguousarray(
            Wv_e[:, hs, :].reshape(NKC, 128, HD2).transpose(1, 0, 2)).astype(BF16)
        m["p_cq"] = np.stack([csq[hs].reshape(HD2),
                              bq[hs].reshape(HD2)]).astype(BF16)
        m["p_ck"] = np.stack([csk[hs].reshape(HD2),
                              bk[hs].reshape(HD2)]).astype(BF16)
        m["p_cv"] = np.stack([csv[hs].reshape(HD2),
                              bv[hs].reshape(HD2)]).astype(BF16)
        in_maps.append(m)
    return in_maps


def kernel(**inputs) -> np.ndarray:
    from concourse.bass_utils import run_bass_kernel_spmd

    if "nc" not in _BUILT:
        _BUILT["nc"] = _build()
    nc = _BUILT["nc"]

    in_maps = _host_prep(inputs)
    res = run_bass_kernel_spmd(nc, in_maps, core_ids=list(range(N_CORES)))

    out = np.empty((B, T, C), np.float32)
    for r in range(N_CORES):
        b_r, s_r = divmod(r, N_CORES // B)
        out[b_r, s_r * TS:(s_r + 1) * TS, :] = res.results[r]["p_out"].T
    return out


# revision 13
# speedup vs baseline: 1.0546x; 1.0546x over previous
"""Trainium2 Bass kernel for a dense transformer block (pre-LN, 16-head causal
attention + 3x FFN), distributed over 8 NeuronCores.

Sharding: tensor-parallel over heads (2 heads/core, both batch elements on
every core) for LN1/QKV/attention; one 8-core AllToAll redistributes the
per-head attention context to token-parallel shards (512 tokens/core) for the
output projection, LN2 and the FFN.  Matmuls run in bf16 with f32 PSUM
accumulation; the residual stream stays f32.

All layouts are transposed ([channel, token]) on chip so every matmul
contracts over the partition dim.  LayerNorm 1 is folded into the QKV weights
(rank-2 correction matmul per accumulation group); LayerNorm 2 is folded into
the first FFN matmul (rank-1 -mu*colsum(W1) correction + per-token 1/std
scale at eviction).

Scheduling notes:
 - the LN1 stats path (bn_stats -> tiny AllGather) is emitted at high
   priority and the bulk x^T/weight DMAs are pushed past it with
   tile_wait_until, so the collective's SDMA traffic isn't stuck behind
   megabytes of queued descriptors
 - attention interleaves the two batches' q-chunks so one chunk's softmax
   drain hides under the other's score/AV stream; softmax normalization is
   deferred into the next round's PE stream
 - every DMA is layed out >=2KB-contiguous per partition on both sides
"""

import numpy as np
import ml_dtypes

B, T, C = 2, 2048, 1024
NH, H = 16, 64
FF = 3 * C
EPS = 1e-6
N_CORES = 8
TT = B * T            # 4096 tokens processed per core (head-parallel phase)
TS = TT // N_CORES    # 512 tokens per core (token-parallel phase)
HPC = NH // N_CORES   # 2 heads per core
HD2 = HPC * H         # 128

BF16 = ml_dtypes.bfloat16

_BUILT = {}

NT = TT // 128        # 32 token tiles
NKC = C // 128        # 8 channel k-tiles
NMF = FF // 128       # 24 ff tiles
NCH = TT // 512       # 8 512-token chunks


def _build():
    import concourse.bacc as bacc
    import concourse.mybir as mybir
    import concourse.tile as tile
    dt = mybir.dt
    alu = mybir.AluOpType
    act = mybir.ActivationFunctionType

    nc = bacc.Bacc("TRN2", target_bir_lowering=False, debug=False,
                   num_devices=N_CORES)

    # ----- kernel I/O (per-core shards) -----
    # layouts chosen so every DMA is contiguous per SBUF partition
    p_x = nc.declare_dram_parameter("p_x", [TT // N_CORES, C], dt.bfloat16, isOutput=False)
    p_xT = nc.declare_dram_parameter("p_xT", [NCH, 128, NKC, 512], dt.bfloat16, isOutput=False)
    p_xTs = nc.declare_dram_parameter("p_xTs", [128, NKC, TS], dt.float32, isOutput=False)
    p_wq = nc.declare_dram_parameter("p_wq", [128, NKC, HD2], dt.bfloat16, isOutput=False)
    p_wk = nc.declare_dram_parameter("p_wk", [128, NKC, HD2], dt.bfloat16, isOutput=False)
    p_wv = nc.declare_dram_parameter("p_wv", [128, NKC, HD2], dt.bfloat16, isOutput=False)
    p_cq = nc.declare_dram_parameter("p_cq", [2, HD2], dt.bfloat16, isOutput=False)
    p_ck = nc.declare_dram_parameter("p_ck", [2, HD2], dt.bfloat16, isOutput=False)
    p_cv = nc.declare_dram_parameter("p_cv", [2, HD2], dt.bfloat16, isOutput=False)
    p_woblk = nc.declare_dram_parameter("p_woblk", [NKC, 128, NKC, 128], dt.bfloat16, isOutput=False)
    p_bo = nc.declare_dram_parameter("p_bo", [1, C], dt.bfloat16, isOutput=False)
    p_w1blk = nc.declare_dram_parameter("p_w1blk", [NMF, 128, NKC, 128], dt.bfloat16, isOutput=False)
    p_b1c = nc.declare_dram_parameter("p_b1c", [128, NMF], dt.float32, isOutput=False)
    p_cs1 = nc.declare_dram_parameter("p_cs1", [1, FF], dt.bfloat16, isOutput=False)
    p_w2blk = nc.declare_dram_parameter("p_w2blk", [NKC, 128, NMF, 128], dt.bfloat16, isOutput=False)
    p_b2 = nc.declare_dram_parameter("p_b2", [1, C], dt.bfloat16, isOutput=False)
    p_maskd = nc.declare_dram_parameter("p_maskd", [128, 128], dt.bfloat16, isOutput=False)
    p_ident = nc.declare_dram_parameter("p_ident", [128, 128], dt.bfloat16, isOutput=False)
    p_out = nc.declare_dram_parameter("p_out", [C, TS], dt.float32, isOutput=True)

    with tile.TileContext(nc, num_cores=N_CORES) as tc:
        with (
            tc.tile_pool(name="persist", bufs=1) as pp,
            tc.tile_pool(name="dram", bufs=1, space="DRAM") as pdram,
        ):
            # ------------- persistent constants & activation tensors -------------
            rows_all = pp.tile([2, TT], dt.bfloat16)
            inv_row = pp.tile([1, TT], dt.bfloat16)
            inv_b = pp.tile([128, TT], dt.bfloat16)
            qT = pp.tile([128, TT], dt.bfloat16)
            kT = pp.tile([128, TT], dt.bfloat16)
            v = pp.tile([128, NT, 2, 65], dt.bfloat16)
            ctxT = pp.tile([128, TT], dt.bfloat16)
            ident = pp.tile([128, 128], dt.bfloat16)
            maskd = pp.tile([128, 128], dt.bfloat16)
            ones_row = pp.tile([1, 512], dt.bfloat16)
            ones128_row = pp.tile([1, 128], dt.bfloat16)
            isc_col = pp.tile([128, 1], dt.bfloat16)   # 1/1024 column for LN2 sums
            cq = pp.tile([2, HD2], dt.bfloat16)
            ck = pp.tile([2, HD2], dt.bfloat16)
            cv = pp.tile([2, HD2], dt.bfloat16)
            wq = pp.tile([128, NKC, HD2], dt.bfloat16)
            wk = pp.tile([128, NKC, HD2], dt.bfloat16)
            wv = pp.tile([128, NKC, HD2], dt.bfloat16)

            cc_in = pdram.tile([N_CORES, 128, TS], dt.bfloat16)
            cc_out = pdram.tile([N_CORES, 128, TS], dt.bfloat16)

            # ---------------- stage A: LN1 stats (sharded) + QKV ----------------
            with (
                tc.tile_pool(name="xtpool", bufs=1) as pxt,
                tc.tile_pool(name="stat", bufs=1) as pst,
                tc.tile_pool(name="apsum", bufs=3, space="PSUM") as pps_a,
                tc.tile_pool(name="apsum1", bufs=1, space="PSUM") as pps_a1,
                tc.tile_pool(name="vtpsum", bufs=2, space="PSUM") as pps_vt,
            ):
                NLT = NT // N_CORES        # 4 local token tiles
                # ---- stats path first, at high priority: nothing bulky may
                # delay the tiny AllGather ----
                with tc.high_priority():
                    nc.sync.dma_start(ident[:], p_ident[:])
                    xla = pst.tile([128, NLT, C], dt.bfloat16)
                    nc.sync.dma_start(
                        xla[:], p_x.ap().rearrange("(i p) c -> p i c", p=128))
                    stats = pst.tile([128, NLT, 2], dt.float32)
                    for i in range(NLT):
                        bnt = pst.tile([128, 2, 6], dt.float32, tag="bnt")
                        nc.vector.bn_stats(bnt[:, 0, :], xla[:, i, 0:512])
                        nc.vector.bn_stats(bnt[:, 1, :], xla[:, i, 512:1024])
                        nc.vector.bn_aggr(stats[:, i, :], bnt[:])

                    # (negmu, std+eps, inv) for the local 512 tokens
                    stat2 = pst.tile([128, NLT, 2], dt.bfloat16)
                    stdf = pst.tile([128, NLT], dt.float32)
                    nc.scalar.activation(stdf[:], stats[:, :, 1], act.Sqrt,
                                         scale=float(C) / (C - 1))
                    nc.vector.tensor_scalar(stdf[:], stdf[:], EPS, None, alu.add)
                    invf = pst.tile([128, NLT], dt.float32)
                    nc.vector.reciprocal(invf[:], stdf[:])
                    nc.vector.tensor_scalar(stat2[:, :, 0], stats[:, :, 0], -1.0,
                                            None, alu.mult)
                    nc.vector.tensor_copy(stat2[:, :, 1], stdf[:])
                    statinv = pst.tile([128, NLT], dt.bfloat16)
                    nc.vector.tensor_copy(statinv[:], invf[:])

                    rows_loc = pst.tile([2, TS], dt.bfloat16)
                    rows_locv = pst.tile([1, TS], dt.bfloat16)
                    for i in range(NLT):
                        pt = pps_a1.tile([2, 128], dt.bfloat16, tag="rowtp")
                        nc.tensor.transpose(pt[:], stat2[:, i, :], ident[:])
                        nc.vector.tensor_copy(rows_loc[:, 128 * i:128 * (i + 1)], pt[:])
                        ptv = pps_a1.tile([1, 128], dt.bfloat16, tag="rowtpv")
                        nc.tensor.transpose(ptv[:], statinv[:, i:i + 1], ident[:])
                        nc.vector.tensor_copy(rows_locv[:, 128 * i:128 * (i + 1)], ptv[:])

                    # all-gather the stat rows (tiny, latency-bound)
                    st_in = pdram.tile([3, TS], dt.bfloat16)
                    st_out = pdram.tile([N_CORES, 3, TS], dt.bfloat16)
                    nc.sync.dma_start(st_in[0:2, :], rows_loc[:])
                    nc.sync.dma_start(st_in[2:3, :], rows_locv[:])
                    nc.gpsimd.collective_compute(
                        "AllGather", alu.bypass,
                        replica_groups=[list(range(N_CORES))],
                        ins=[st_in.opt()],
                        outs=[st_out.opt()],
                    )
                    nc.sync.dma_start(
                        rows_all[:].rearrange("s (r t) -> s r t", r=N_CORES),
                        st_out[:, 0:2, :].rearrange("r s t -> s r t"))
                    nc.sync.dma_start(
                        inv_row[:].rearrange("s (r t) -> s r t", r=N_CORES),
                        st_out[:, 2:3, :].rearrange("r s t -> s r t"))

                # ---- weights: modeled AFTER the stats collective (the
                # model prices any collective at >=15us) so the sync
                # sequencer physically holds the bulk flood until the mesh
                # completes -- keeps the wire clear for the collective and
                # makes every core's timeline deterministic ----
                nc.vector.memset(ones_row[:], 1.0)
                nc.vector.memset(ones128_row[:], 1.0)
                nc.vector.memset(isc_col[:], 1.0 / C)
                with tc.tile_wait_until(0.027):
                    nc.sync.dma_start(wq[:], p_wq[:])
                    nc.sync.dma_start(wk[:], p_wk[:])
                    nc.sync.dma_start(wv[:], p_wv[:])
                    nc.sync.dma_start(cq[:], p_cq[:])
                    nc.sync.dma_start(ck[:], p_ck[:])
                    nc.sync.dma_start(cv[:], p_cv[:])
                    nc.sync.dma_start(maskd[:], p_maskd[:])

                xT = pxt.tile([128, NCH, NKC, 512], dt.bfloat16)
                for ch in range(NCH):
                    with tc.tile_wait_until(0.028 + 0.0025 * ch):
                        nc.sync.dma_start(xT[:, ch], p_xT[ch])

                vT = pxt.tile([128, TT], dt.bfloat16)
                nc.vector.memset(v[:, :, :, 64], 1.0)
                for ch in range(NCH):
                    sl = slice(512 * ch, 512 * (ch + 1))
                    for (nm, w, cw, dst) in (("q", wq, cq, qT), ("k", wk, ck, kT),
                                             ("v", wv, cv, vT)):
                        ps = pps_a.tile([128, 512], dt.float32,
                                        name=f"ps{nm}", tag="qkv")
                        for k in range(NKC):
                            nc.tensor.matmul(ps[:], w[:, k, :], xT[:, ch, k, :],
                                             start=(k == 0), stop=False)
                        if nm == "q":
                            # inv broadcast for this chunk: after the q mains
                            # (PE runway for the stats collective), before any
                            # eviction reads it
                            pb = pps_a1.tile([128, 512], dt.float32, tag="invb")
                            nc.tensor.matmul(pb[:], ones128_row[:],
                                             inv_row[0:1, sl],
                                             start=True, stop=True)
                            nc.scalar.copy(inv_b[:, sl], pb[:])
                        nc.tensor.matmul(ps[:], cw[:], rows_all[0:2, sl],
                                         start=False, stop=True)
                        nc.vector.tensor_tensor(dst[:, sl], ps[:], inv_b[:, sl],
                                                alu.mult)
                    # v_aug [s, tile, head, 65] via PE transpose, interleaved
                    # with the QKV matmul stream to keep the PE warm
                    for i in range(4 * ch, 4 * ch + 4):
                        pvt = pps_vt.tile([128, 128], dt.bfloat16, tag="vtp")
                        nc.tensor.transpose(pvt[:], vT[:, 128 * i:128 * (i + 1)],
                                            ident[:])
                        if i % 2 == 0:
                            nc.scalar.copy(v[:, i, :, 0:64],
                                           pvt[:].rearrange("p (h d) -> p h d", h=2))
                        else:
                            nc.vector.tensor_copy(v[:, i, :, 0:64],
                                                  pvt[:].rearrange("p (h d) -> p h d", h=2))

            # ---------------- stage B: attention ----------------
            # two batches interleaved: batch b0's chunk and b1's chunk of the
            # same size run j-step-by-j-step so each hides the other's
            # softmax/normalize bubbles.
            with (
                tc.tile_pool(name="exps", bufs=4) as pexp,
                tc.tile_pool(name="attsb", bufs=2) as pat,
                tc.tile_pool(name="scpsum", bufs=4, space="PSUM") as pps_sc,
                tc.tile_pool(name="ctxpsum", bufs=1, space="PSUM") as pps_ctx,
            ):
                def emit_norm(stt):
                    pcs_, zinvb_, gsl_, cidx_ = stt
                    pzb = pps_sc.tile([128, 512], dt.float32, name="pzb", tag="sc")
                    for h in range(2):
                        nc.tensor.matmul(pzb[64 * h:64 * (h + 1), :],
                                         ones128_row[0:1, 0:64], zinvb_[h][:],
                                         start=True, stop=True)
                    zb = pat.tile([128, 512], dt.bfloat16, tag="zbs")
                    nc.vector.tensor_copy(zb[:], pzb[:])
                    for h in range(2):
                        nc.vector.tensor_tensor(
                            ctxT[64 * h:64 * (h + 1), gsl_],
                            pcs_[h][0:64, :], zb[64 * h:64 * (h + 1), :],
                            alu.mult)
                    nc.sync.dma_start(cc_in[cidx_], ctxT[:, gsl_])

                def emit_tail(prev):
                    # final (diagonal) AV matmuls of the previous round, then
                    # its 1/Z chains on the vector engine
                    for b in range(B):
                        for h in range(2):
                            nc.tensor.matmul(
                                prev["pc"][b][h][:],
                                v[:, prev["vidx"][b], h, :],
                                prev["et_last"][b][h][:],
                                start=False, stop=True)
                    for b in range(B):
                        zinvb = []
                        for h in range(2):
                            zrow = pat.tile([1, 512], dt.float32,
                                            name=f"zrow{b}{h}", tag=f"z{b}{h}")
                            nc.vector.tensor_copy(zrow[:], prev["pc"][b][h][64:65, :])
                            zinv = pat.tile([1, 512], dt.float32,
                                            name=f"zinv{b}{h}", tag=f"zi{b}{h}")
                            nc.vector.reciprocal_approx_fast(zinv[:], zrow[:])
                            zib = pat.tile([1, 512], dt.bfloat16,
                                           name=f"zinvb{b}{h}", tag=f"zib{b}{h}")
                            nc.vector.tensor_copy(zib[:], zinv[:])
                            zinvb.append(zib)
                        prev["zinvb"][b] = zinvb

                prev = None
                for qt in range(T // 512):
                    nj = 4 * qt + 4
                    gsl = {b: slice(b * T + 512 * qt, b * T + 512 * qt + 512)
                           for b in range(B)}
                    pc = None
                    ets = {b: [] for b in range(B)}
                    for j in range(nj):
                        for b in range(B):
                            st = b * (T // 128) + j   # global s-tile index
                            et2 = []
                            for h in range(2):
                                hsl = slice(64 * h, 64 * (h + 1))
                                ps = pps_sc.tile([128, 512], dt.float32,
                                                 name="ps", tag="sc")
                                nc.tensor.matmul(
                                    ps[:], kT[hsl, 128 * st:128 * (st + 1)],
                                    qT[hsl, gsl[b]], start=True, stop=True)
                                et = pexp.tile([128, 512], dt.bfloat16,
                                               name=f"et{b}{h}", tag=f"et{b}{h}")
                                if j >= nj - 4:
                                    off = j - (nj - 4)
                                    if off > 0:
                                        nc.gpsimd.memset(et[:, 0:128 * off], 0.0)
                                    nc.scalar.activation(
                                        et[:, 128 * off:512], ps[:, 128 * off:512],
                                        act.Exp, scale=1.0 / float(np.sqrt(H)))
                                    nc.gpsimd.tensor_tensor(
                                        et[:, 128 * off:128 * (off + 1)],
                                        et[:, 128 * off:128 * (off + 1)],
                                        maskd[:], alu.mult)
                                else:
                                    nc.scalar.activation(et[:], ps[:], act.Exp,
                                                         scale=1.0 / float(np.sqrt(H)))
                                et2.append(et)
                            ets[b].append(et2)
                        if j == 0 and prev is not None:
                            emit_tail(prev)
                        if j == 1:
                            if prev is not None:
                                for b in range(B):
                                    emit_norm((prev["pc"][b], prev["zinvb"][b],
                                               prev["gsl"][b], prev["cidx"][b]))
                                prev = None
                            pc = {b: [pps_ctx.tile([65, 512], dt.float32,
                                                   name=f"pc{b}{h}", tag=f"ctx{b}{h}")
                                      for h in range(2)] for b in range(B)}
                        if j > 0:
                            for b in range(B):
                                for h in range(2):
                                    nc.tensor.matmul(
                                        pc[b][h][:], v[:, b * (T // 128) + j - 1, h, :],
                                        ets[b][j - 1][h][:],
                                        start=(j - 1 == 0), stop=False)
                    prev = {"pc": pc,
                            "et_last": {b: ets[b][nj - 1] for b in range(B)},
                            "vidx": {b: b * (T // 128) + nj - 1 for b in range(B)},
                            "gsl": gsl,
                            "cidx": {b: b * (T // 512) + qt for b in range(B)},
                            "zinvb": {}}
                emit_tail(prev)
                for b in range(B):
                    emit_norm((prev["pc"][b], prev["zinvb"][b],
                               prev["gsl"][b], prev["cidx"][b]))
                prev = None

            # ---------------- AllToAll: heads -> tokens ----------------
            nc.gpsimd.collective_compute(
                "AllToAll", alu.bypass,
                replica_groups=[list(range(N_CORES))],
                ins=[cc_in.opt()],
                outs=[cc_out.opt()],
            )

            # ---------------- stage C: Wo + LN2 + FFN ----------------
            with (
                tc.tile_pool(name="postsb", bufs=1) as pq,
                tc.tile_pool(name="wstream", bufs=3) as pw,
                tc.tile_pool(name="evict", bufs=3) as pev,
                tc.tile_pool(name="ln2tmp", bufs=1) as pl2,
                tc.tile_pool(name="ffpsum", bufs=2, space="PSUM") as pps_ff,
                tc.tile_pool(name="cpsum", bufs=1, space="PSUM") as pps_c,
            ):
                ctxF = pq.tile([128, NKC, TS], dt.bfloat16)
                nc.sync.dma_start(ctxF[:],
                                  cc_out[:].rearrange("c p t -> p c t"))

                bo = pq.tile([1, C], dt.bfloat16)
                b2 = pq.tile([1, C], dt.bfloat16)
                b1c = pq.tile([128, NMF], dt.float32)
                cs1 = pq.tile([1, FF], dt.bfloat16)
                xTs = pq.tile([128, NKC, TS], dt.float32)
                with tc.tile_wait_until(0.12):
                    nc.sync.dma_start(bo[:], p_bo[:])
                    nc.sync.dma_start(b2[:], p_b2[:])
                    nc.sync.dma_start(b1c[:], p_b1c[:])
                    nc.sync.dma_start(cs1[:], p_cs1[:])
                    nc.sync.dma_start(xTs[:], p_xTs[:])

                # Wo + residual; LN2 column sums interleaved per block
                r2T = pq.tile([128, NKC, TS], dt.float32)
                r2b = pl2.tile([128, NKC, TS], dt.bfloat16)
                sq = pl2.tile([128, NKC, TS], dt.bfloat16)
                ps1 = pps_c.tile([1, TS], dt.float32, tag="s1")
                ps2 = pps_c.tile([1, TS], dt.float32, tag="s2")
                for mc in range(NKC):
                    wo_blk = pw.tile([128, NKC, 128], dt.bfloat16, tag="wo")
                    with tc.tile_wait_until(0.12):
                        nc.sync.dma_start(wo_blk[:], p_woblk[mc])
                    ps = pps_ff.tile([128, TS], dt.float32, tag="ff")
                    for k in range(NKC):
                        nc.tensor.matmul(ps[:], wo_blk[:, k, :], ctxF[:, k, :],
                                         start=(k == 0), stop=False)
                    nc.tensor.matmul(ps[:], bo[0:1, 128 * mc:128 * (mc + 1)],
                                     ones_row[:], start=False, stop=True)
                    nc.vector.tensor_tensor(r2T[:, mc, :], ps[:], xTs[:, mc, :],
                                            alu.add)
                    nc.scalar.copy(r2b[:, mc, :], r2T[:, mc, :])
                    nc.vector.tensor_tensor(sq[:, mc, :], r2b[:, mc, :],
                                            r2b[:, mc, :], alu.mult)
                    nc.tensor.matmul(ps1[:], isc_col[:], r2b[:, mc, :],
                                     start=(mc == 0), stop=(mc == NKC - 1))
                    nc.tensor.matmul(ps2[:], isc_col[:], sq[:, mc, :],
                                     start=(mc == 0), stop=(mc == NKC - 1))

                # ---- LN2 scalars: -mu row and broadcast 1/(std+eps) ----
                muf = pl2.tile([1, TS], dt.float32)
                nc.vector.tensor_copy(muf[:], ps1[:])
                negmu2 = pl2.tile([1, TS], dt.bfloat16)
                nc.vector.tensor_scalar(negmu2[:], muf[:], -1.0, None, alu.mult)
                varf = pl2.tile([1, TS], dt.float32)
                nc.vector.tensor_tensor(varf[:], muf[:], muf[:], alu.mult)
                nc.vector.tensor_tensor(varf[:], ps2[:], varf[:], alu.subtract)
                stdf2 = pl2.tile([1, TS], dt.float32)
                nc.scalar.activation(stdf2[:], varf[:], act.Sqrt,
                                     scale=float(C) / (C - 1))
                nc.vector.tensor_scalar(stdf2[:], stdf2[:], EPS, None, alu.add)
                inv2 = pl2.tile([1, TS], dt.float32)
                nc.vector.reciprocal_approx_fast(inv2[:], stdf2[:])
                inv2row = pl2.tile([1, TS], dt.bfloat16)
                nc.vector.tensor_copy(inv2row[:], inv2[:])
                pib = pps_c.tile([128, TS], dt.float32, tag="bcast")
                nc.tensor.matmul(pib[:], ones128_row[:], inv2row[:],
                                 start=True, stop=True)
                i2b = pl2.tile([128, TS], dt.bfloat16)
                nc.scalar.copy(i2b[:], pib[:])

                # ---- FFN (LN2 folded into the W1 matmul) ----
                hT = pq.tile([128, NMF, TS], dt.bfloat16)
                for mf in range(NMF):
                    w1_blk = pw.tile([128, NKC, 128], dt.bfloat16, tag="w1")
                    with tc.tile_wait_until(0.15):
                        nc.sync.dma_start(w1_blk[:], p_w1blk[mf])
                    ps = pps_ff.tile([128, TS], dt.float32, tag="ff")
                    for k in range(NKC):
                        nc.tensor.matmul(ps[:], w1_blk[:, k, :], r2b[:, k, :],
                                         start=(k == 0), stop=False)
                    nc.tensor.matmul(ps[:], cs1[0:1, 128 * mf:128 * (mf + 1)],
                                     negmu2[:], start=False, stop=True)
                    tmp = pev.tile([128, TS], dt.bfloat16, tag="htmp")
                    nc.vector.tensor_tensor(tmp[:], ps[:], i2b[:], alu.mult)
                    nc.vector.tensor_scalar(hT[:, mf, :], tmp[:], b1c[:, mf:mf + 1],
                                            0.0, alu.add, alu.max)

                for mc in range(NKC):
                    w2_blk = pw.tile([128, NMF, 128], dt.bfloat16, tag="w2")
                    with tc.tile_wait_until(0.18):
                        nc.sync.dma_start(w2_blk[:], p_w2blk[mc])
                    ps = pps_ff.tile([128, TS], dt.float32, tag="ff")
                    for k in range(NMF):
                        nc.tensor.matmul(ps[:], w2_blk[:, k, :], hT[:, k, :],
                                         start=(k == 0), stop=False)
                    nc.tensor.matmul(ps[:], b2[0:1, 128 * mc:128 * (mc + 1)],
                                     ones_row[:], start=False, stop=True)
                    ot = pev.tile([128, TS], dt.float32, tag="ot")
                    nc.vector.tensor_tensor(ot[:], ps[:], r2T[:, mc, :], alu.add)
                    nc.sync.dma_start(p_out[128 * mc:128 * (mc + 1), :], ot[:])

    nc.compile()
    return nc


def _host_prep(inputs):
    """Fold layernorm affine params into weights; build per-core input maps."""
    x = np.asarray(inputs["x"], np.float32)
    Wq = np.asarray(inputs["Wq"], np.float32)
    Wk = np.asarray(inputs["Wk"], np.float32)
    Wv = np.asarray(inputs["Wv"], np.float32)
    Wo = np.asarray(inputs["Wo"], np.float32)
    bo = np.asarray(inputs["bo"], np.float32)
    W1 = np.asarray(inputs["W1"], np.float32)
    b1 = np.asarray(inputs["b1"], np.float32)
    W2 = np.asarray(inputs["W2"], np.float32)
    b2 = np.asarray(inputs["b2"], np.float32)
    g1 = np.asarray(inputs["g1"], np.float32)
    be1 = np.asarray(inputs["be1"], np.float32)
    g2 = np.asarray(inputs["g2"], np.float32)
    be2 = np.asarray(inputs["be2"], np.float32)

    xf = x.reshape(TT, C)                      # both batches stacked
    xT = np.ascontiguousarray(xf.T)            # [C, TT]
    # chunk-major, partition-contiguous: [NCH, 128, NKC, 512]
    xT_blk = np.ascontiguousarray(
        xT.reshape(NKC, 128, NCH, 512).transpose(2, 1, 0, 3))

    def fold_qkv(W):
        Weff = g1[:, None] * W                  # [NH, C, H] with g1 on C
        Weff = np.ascontiguousarray(np.transpose(Weff, (1, 0, 2)))  # [C, NH, H]
        bias = np.einsum("c,hck->hk", be1, W)   # [NH, H]
        colsum = Weff.sum(axis=0)               # [NH, H]
        return Weff, bias, colsum

    Wq_e, bq, csq = fold_qkv(Wq)
    Wk_e, bk, csk = fold_qkv(Wk)
    Wv_e, bv, csv = fold_qkv(Wv)

    woT = np.ascontiguousarray(Wo.T)            # [NH*H, C]
    w1T = np.ascontiguousarray(g2[:, None] * W1.T)   # [C, FF]
    b1_eff = b1 + be2 @ W1.T                         # [FF]
    cs1 = w1T.sum(axis=0)                            # [FF]
    w2T = np.ascontiguousarray(W2.T)            # [FF, C]

    # blocked weights [mblk, 128, K/128, 128], contiguous per partition
    woblk = np.ascontiguousarray(
        woT.reshape(NKC, 128, NKC, 128).transpose(2, 1, 0, 3))
    w1blk = np.ascontiguousarray(
        w1T.reshape(NKC, 128, NMF, 128).transpose(2, 1, 0, 3))
    w2blk = np.ascontiguousarray(
        w2T.reshape(NMF, 128, NKC, 128).transpose(2, 1, 0, 3))

    tq = np.arange(128)[None, :]
    s = np.arange(128)[:, None]
    maskd = (s <= tq).astype(BF16)

    x_bf = xf.astype(BF16)
    shared = {
        "p_xT": xT_blk.astype(BF16),
        "p_woblk": woblk.astype(BF16),
        "p_bo": bo[None, :].astype(BF16),
        "p_w1blk": w1blk.astype(BF16),
        "p_b1c": np.ascontiguousarray(
            b1_eff.reshape(NMF, 128).T).astype(np.float32),
        "p_cs1": cs1[None, :].astype(BF16),
        "p_w2blk": w2blk.astype(BF16),
        "p_b2": b2[None, :].astype(BF16),
        "p_maskd": maskd,
        "p_ident": np.eye(128, dtype=np.float32).astype(BF16),
    }

    in_maps = []
    for r in range(N_CORES):
        h0 = HPC * r
        hs = slice(h0, h0 + HPC)
        b_r, s_r = divmod(r, N_CORES // B)
        tok = slice(s_r * TS, (s_r + 1) * TS)
        xTs = np.ascontiguousarray(
            x[b_r].T[:, tok].reshape(NKC, 128, TS).transpose(1, 0, 2))
        m = dict(shared)
        m["p_x"] = x_bf[r * (TT // N_CORES):(r + 1) * (TT // N_CORES), :]
        m["p_xTs"] = xTs.astype(np.float32)
        m["p_wq"] = np.ascontiguousarray(
            Wq_e[:, hs, :].reshape(NKC, 128, HD2).transpose(1, 0, 2)).astype(BF16)
        m["p_wk"] = np.ascontiguousarray(
            Wk_e[:, hs, :].reshape(NKC, 128, HD2).transpose(1, 0, 2)).astype(BF16)
        m["p_wv"] = np.ascontiguousarray(
            Wv_e[:, hs, :].reshape(NKC, 128, HD2).transpose(1, 0, 2)).astype(BF16)
        m["p_cq"] = np.stack([csq[hs].reshape(HD2),
                              bq[hs].reshape(HD2)]).astype(BF16)
        m["p_ck"] = np.stack([csk[hs].reshape(HD2),
                              bk[hs].reshape(HD2)]).astype(BF16)
        m["p_cv"] = np.stack([csv[hs].reshape(HD2),
                              bv[hs].reshape(HD2)]).astype(BF16)
        in_maps.append(m)
    return in_maps


def kernel(**inputs) -> np.ndarray:
    from concourse.bass_utils import run_bass_kernel_spmd

    if "nc" not in _BUILT:
        _BUILT["nc"] = _build()
    nc = _BUILT["nc"]

    in_maps = _host_prep(inputs)
    res = run_bass_kernel_spmd(nc, in_maps, core_ids=list(range(N_CORES)))

    out = np.empty((B, T, C), np.float32)
    for r in range(N_CORES):
        b_r, s_r = divmod(r, N_CORES // B)
        out[b_r, s_r * TS:(s_r + 1) * TS, :] = res.results[r]["p_out"].T
    return out


# revision 15
# speedup vs baseline: 1.0901x; 1.0336x over previous
"""Trainium2 Bass kernel for a dense transformer block (pre-LN, 16-head causal
attention + 3x FFN), distributed over 8 NeuronCores.

Sharding: tensor-parallel over heads (2 heads/core, both batch elements on
every core) for LN1/QKV/attention; one 8-core AllToAll redistributes the
per-head attention context to token-parallel shards (512 tokens/core) for the
output projection, LN2 and the FFN.  Matmuls run in bf16 with f32 PSUM
accumulation; the residual stream stays f32.

All layouts are transposed ([channel, token]) on chip so every matmul
contracts over the partition dim.  LayerNorm 1 is folded into the QKV weights
(rank-2 correction matmul per accumulation group); LayerNorm 2 is folded into
the first FFN matmul (rank-1 -mu*colsum(W1) correction + per-token 1/std
scale at eviction).

Scheduling notes:
 - the LN1 stats path (bn_stats -> tiny AllGather) is emitted at high
   priority and the bulk x^T/weight DMAs are pushed past it with
   tile_wait_until, so the collective's SDMA traffic isn't stuck behind
   megabytes of queued descriptors
 - attention interleaves the two batches' q-chunks so one chunk's softmax
   drain hides under the other's score/AV stream; softmax normalization is
   deferred into the next round's PE stream
 - every DMA is layed out >=2KB-contiguous per partition on both sides
"""

import numpy as np
import ml_dtypes

B, T, C = 2, 2048, 1024
NH, H = 16, 64
FF = 3 * C
EPS = 1e-6
N_CORES = 8
TT = B * T            # 4096 tokens processed per core (head-parallel phase)
TS = TT // N_CORES    # 512 tokens per core (token-parallel phase)
HPC = NH // N_CORES   # 2 heads per core
HD2 = HPC * H         # 128

BF16 = ml_dtypes.bfloat16

_BUILT = {}

NT = TT // 128        # 32 token tiles
NKC = C // 128        # 8 channel k-tiles
NMF = FF // 128       # 24 ff tiles
NCH = TT // 512       # 8 512-token chunks


def _build():
    import concourse.bacc as bacc
    import concourse.mybir as mybir
    import concourse.tile as tile
    dt = mybir.dt
    alu = mybir.AluOpType
    act = mybir.ActivationFunctionType

    nc = bacc.Bacc("TRN2", target_bir_lowering=False, debug=False,
                   num_devices=N_CORES)

    # ----- kernel I/O (per-core shards) -----
    # layouts chosen so every DMA is contiguous per SBUF partition
    p_x = nc.declare_dram_parameter("p_x", [TT // N_CORES, C], dt.bfloat16, isOutput=False)
    p_xT = nc.declare_dram_parameter("p_xT", [NCH, 128, NKC, 512], dt.bfloat16, isOutput=False)
    p_xTs = nc.declare_dram_parameter("p_xTs", [128, NKC, TS], dt.float32, isOutput=False)
    p_wq = nc.declare_dram_parameter("p_wq", [128, NKC, HD2], dt.bfloat16, isOutput=False)
    p_wk = nc.declare_dram_parameter("p_wk", [128, NKC, HD2], dt.bfloat16, isOutput=False)
    p_wv = nc.declare_dram_parameter("p_wv", [128, NKC, HD2], dt.bfloat16, isOutput=False)
    p_cq = nc.declare_dram_parameter("p_cq", [2, HD2], dt.bfloat16, isOutput=False)
    p_ck = nc.declare_dram_parameter("p_ck", [2, HD2], dt.bfloat16, isOutput=False)
    p_cv = nc.declare_dram_parameter("p_cv", [2, HD2], dt.bfloat16, isOutput=False)
    p_woblk = nc.declare_dram_parameter("p_woblk", [NKC, 128, NKC, 128], dt.bfloat16, isOutput=False)
    p_boc = nc.declare_dram_parameter("p_boc", [128, NKC], dt.float32, isOutput=False)
    p_w1blk = nc.declare_dram_parameter("p_w1blk", [NMF, 128, NKC, 128], dt.bfloat16, isOutput=False)
    p_b1c = nc.declare_dram_parameter("p_b1c", [128, NMF], dt.float32, isOutput=False)
    p_cs1 = nc.declare_dram_parameter("p_cs1", [1, FF], dt.bfloat16, isOutput=False)
    p_w2blk = nc.declare_dram_parameter("p_w2blk", [NKC, 128, NMF, 128], dt.bfloat16, isOutput=False)
    p_b2c = nc.declare_dram_parameter("p_b2c", [128, NKC], dt.float32, isOutput=False)
    p_maskd = nc.declare_dram_parameter("p_maskd", [128, 128], dt.bfloat16, isOutput=False)
    p_ident = nc.declare_dram_parameter("p_ident", [128, 128], dt.bfloat16, isOutput=False)
    p_out = nc.declare_dram_parameter("p_out", [C, TS], dt.float32, isOutput=True)

    with tile.TileContext(nc, num_cores=N_CORES) as tc:
        with (
            tc.tile_pool(name="persist", bufs=1) as pp,
            tc.tile_pool(name="dram", bufs=1, space="DRAM") as pdram,
        ):
            # ------------- persistent constants & activation tensors -------------
            rows_all = pp.tile([2, TT], dt.bfloat16)
            inv_row = pp.tile([1, TT], dt.bfloat16)
            inv_b = pp.tile([128, TT], dt.bfloat16)
            qT = pp.tile([128, TT], dt.bfloat16)
            kT = pp.tile([128, TT], dt.bfloat16)
            v = pp.tile([128, NT, 2, 65], dt.bfloat16)
            ctxT = pp.tile([128, TT], dt.bfloat16)
            ident = pp.tile([128, 128], dt.bfloat16)
            maskd = pp.tile([128, 128], dt.bfloat16)
            ones128_row = pp.tile([1, 128], dt.bfloat16)
            isc_col = pp.tile([128, 1], dt.bfloat16)   # 1/1024 column for LN2 sums
            cq = pp.tile([2, HD2], dt.bfloat16)
            ck = pp.tile([2, HD2], dt.bfloat16)
            cv = pp.tile([2, HD2], dt.bfloat16)
            wq = pp.tile([128, NKC, HD2], dt.bfloat16)
            wk = pp.tile([128, NKC, HD2], dt.bfloat16)
            wv = pp.tile([128, NKC, HD2], dt.bfloat16)

            cc_in = pdram.tile([N_CORES, 128, TS], dt.bfloat16)
            cc_out = pdram.tile([N_CORES, 128, TS], dt.bfloat16)

            # ---------------- stage A: LN1 stats (sharded) + QKV ----------------
            with (
                tc.tile_pool(name="xtpool", bufs=1) as pxt,
                tc.tile_pool(name="stat", bufs=1) as pst,
                tc.tile_pool(name="apsum", bufs=3, space="PSUM") as pps_a,
                tc.tile_pool(name="apsum1", bufs=1, space="PSUM") as pps_a1,
                tc.tile_pool(name="vtpsum", bufs=2, space="PSUM") as pps_vt,
            ):
                NLT = NT // N_CORES        # 4 local token tiles
                # ---- stats path first, at high priority: nothing bulky may
                # delay the tiny AllGather ----
                with tc.high_priority():
                    nc.sync.dma_start(ident[:], p_ident[:])
                    xla = pst.tile([128, NLT, C], dt.bfloat16)
                    nc.sync.dma_start(
                        xla[:, 0:2, :],
                        p_x.ap()[0:256, :].rearrange("(i p) c -> p i c", p=128))
                    nc.sync.dma_start(
                        xla[:, 2:4, :],
                        p_x.ap()[256:512, :].rearrange("(i p) c -> p i c", p=128))
                    stats = pst.tile([128, NLT, 2], dt.float32)
                    for i in range(NLT):
                        bnt = pst.tile([128, 2, 6], dt.float32, tag="bnt")
                        nc.vector.bn_stats(bnt[:, 0, :], xla[:, i, 0:512])
                        nc.vector.bn_stats(bnt[:, 1, :], xla[:, i, 512:1024])
                        nc.vector.bn_aggr(stats[:, i, :], bnt[:])

                    # (negmu, std+eps, inv) for the local 512 tokens
                    stat2 = pst.tile([128, NLT, 2], dt.bfloat16)
                    stdf = pst.tile([128, NLT], dt.float32)
                    nc.scalar.activation(stdf[:], stats[:, :, 1], act.Sqrt,
                                         scale=float(C) / (C - 1))
                    nc.vector.tensor_scalar(stdf[:], stdf[:], EPS, None, alu.add)
                    invf = pst.tile([128, NLT], dt.float32)
                    nc.vector.reciprocal(invf[:], stdf[:])
                    nc.vector.tensor_scalar(stat2[:, :, 0], stats[:, :, 0], -1.0,
                                            None, alu.mult)
                    nc.vector.tensor_copy(stat2[:, :, 1], stdf[:])
                    statinv = pst.tile([128, NLT], dt.bfloat16)
                    nc.vector.tensor_copy(statinv[:], invf[:])

                    rows_loc = pst.tile([2, TS], dt.bfloat16)
                    rows_locv = pst.tile([1, TS], dt.bfloat16)
                    for i in range(NLT):
                        pt = pps_a1.tile([2, 128], dt.bfloat16, tag="rowtp")
                        nc.tensor.transpose(pt[:], stat2[:, i, :], ident[:])
                        nc.vector.tensor_copy(rows_loc[:, 128 * i:128 * (i + 1)], pt[:])
                        ptv = pps_a1.tile([1, 128], dt.bfloat16, tag="rowtpv")
                        nc.tensor.transpose(ptv[:], statinv[:, i:i + 1], ident[:])
                        nc.vector.tensor_copy(rows_locv[:, 128 * i:128 * (i + 1)], ptv[:])

                    # all-gather the stat rows (tiny, latency-bound)
                    st_in = pdram.tile([3, TS], dt.bfloat16)
                    st_out = pdram.tile([N_CORES, 3, TS], dt.bfloat16)
                    nc.sync.dma_start(st_in[0:2, :], rows_loc[:])
                    nc.sync.dma_start(st_in[2:3, :], rows_locv[:])
                    nc.gpsimd.collective_compute(
                        "AllGather", alu.bypass,
                        replica_groups=[list(range(N_CORES))],
                        ins=[st_in.opt()],
                        outs=[st_out.opt()],
                    )
                    nc.sync.dma_start(
                        rows_all[:].rearrange("s (r t) -> s r t", r=N_CORES),
                        st_out[:, 0:2, :].rearrange("r s t -> s r t"))
                    nc.sync.dma_start(
                        inv_row[:].rearrange("s (r t) -> s r t", r=N_CORES),
                        st_out[:, 2:3, :].rearrange("r s t -> s r t"))

                # ---- weights: modeled AFTER the stats collective (the
                # model prices any collective at >=15us) so the sync
                # sequencer physically holds the bulk flood until the mesh
                # completes -- keeps the wire clear for the collective and
                # makes every core's timeline deterministic ----
                nc.vector.memset(ones128_row[:], 1.0)
                nc.vector.memset(isc_col[:], 1.0 / C)
                with tc.tile_wait_until(0.032):
                    nc.sync.dma_start(wq[:], p_wq[:])
                    nc.sync.dma_start(wk[:], p_wk[:])
                    nc.sync.dma_start(wv[:], p_wv[:])
                    nc.sync.dma_start(cq[:], p_cq[:])
                    nc.sync.dma_start(ck[:], p_ck[:])
                    nc.sync.dma_start(cv[:], p_cv[:])
                    nc.sync.dma_start(maskd[:], p_maskd[:])

                xT = pxt.tile([128, NCH, NKC, 512], dt.bfloat16)
                for ch in range(NCH):
                    with tc.tile_wait_until(0.034 + 0.002 * ch):
                        nc.sync.dma_start(xT[:, ch], p_xT[ch])

                vT = pxt.tile([128, TT], dt.bfloat16)
                nc.vector.memset(v[:, :, :, 64], 1.0)
                for ch in range(NCH):
                    sl = slice(512 * ch, 512 * (ch + 1))
                    for (nm, w, cw, dst) in (("q", wq, cq, qT), ("k", wk, ck, kT),
                                             ("v", wv, cv, vT)):
                        ps = pps_a.tile([128, 512], dt.float32,
                                        name=f"ps{nm}", tag="qkv")
                        for k in range(NKC):
                            nc.tensor.matmul(ps[:], w[:, k, :], xT[:, ch, k, :],
                                             start=(k == 0), stop=False)
                        if nm == "q":
                            # inv broadcast for this chunk: after the q mains
                            # (PE runway for the stats collective), before any
                            # eviction reads it
                            pb = pps_a1.tile([128, 512], dt.float32, tag="invb")
                            nc.tensor.matmul(pb[:], ones128_row[:],
                                             inv_row[0:1, sl],
                                             start=True, stop=True)
                            nc.scalar.copy(inv_b[:, sl], pb[:])
                        nc.tensor.matmul(ps[:], cw[:], rows_all[0:2, sl],
                                         start=False, stop=True)
                        nc.vector.tensor_tensor(dst[:, sl], ps[:], inv_b[:, sl],
                                                alu.mult)
                    # v_aug [s, tile, head, 65] via PE transpose, interleaved
                    # with the QKV matmul stream to keep the PE warm
                    for i in range(4 * ch, 4 * ch + 4):
                        pvt = pps_vt.tile([128, 128], dt.bfloat16, tag="vtp")
                        nc.tensor.transpose(pvt[:], vT[:, 128 * i:128 * (i + 1)],
                                            ident[:])
                        if i % 2 == 0:
                            nc.scalar.copy(v[:, i, :, 0:64],
                                           pvt[:].rearrange("p (h d) -> p h d", h=2))
                        else:
                            nc.vector.tensor_copy(v[:, i, :, 0:64],
                                                  pvt[:].rearrange("p (h d) -> p h d", h=2))

            # ---------------- stage B: attention ----------------
            # two batches interleaved: batch b0's chunk and b1's chunk of the
            # same size run j-step-by-j-step so each hides the other's
            # softmax/normalize bubbles.
            with (
                tc.tile_pool(name="exps", bufs=4) as pexp,
                tc.tile_pool(name="attsb", bufs=2) as pat,
                tc.tile_pool(name="scpsum", bufs=4, space="PSUM") as pps_sc,
                tc.tile_pool(name="ctxpsum", bufs=1, space="PSUM") as pps_ctx,
            ):
                def emit_norm(stt):
                    pcs_, zinvb_, gsl_, cidx_ = stt
                    pzb = pps_sc.tile([128, 512], dt.float32, name="pzb", tag="sc")
                    for h in range(2):
                        nc.tensor.matmul(pzb[64 * h:64 * (h + 1), :],
                                         ones128_row[0:1, 0:64], zinvb_[h][:],
                                         start=True, stop=True)
                    zb = pat.tile([128, 512], dt.bfloat16, tag="zbs")
                    nc.vector.tensor_copy(zb[:], pzb[:])
                    for h in range(2):
                        nc.vector.tensor_tensor(
                            ctxT[64 * h:64 * (h + 1), gsl_],
                            pcs_[h][0:64, :], zb[64 * h:64 * (h + 1), :],
                            alu.mult)
                    nc.sync.dma_start(cc_in[cidx_], ctxT[:, gsl_])

                def emit_tail(prev):
                    # final (diagonal) AV matmuls of the previous round, then
                    # its 1/Z chains on the vector engine
                    for b in range(B):
                        for h in range(2):
                            nc.tensor.matmul(
                                prev["pc"][b][h][:],
                                v[:, prev["vidx"][b], h, :],
                                prev["et_last"][b][h][:],
                                start=False, stop=True)
                    for b in range(B):
                        zinvb = []
                        for h in range(2):
                            zrow = pat.tile([1, 512], dt.float32,
                                            name=f"zrow{b}{h}", tag=f"z{b}{h}")
                            nc.vector.tensor_copy(zrow[:], prev["pc"][b][h][64:65, :])
                            zinv = pat.tile([1, 512], dt.float32,
                                            name=f"zinv{b}{h}", tag=f"zi{b}{h}")
                            nc.vector.reciprocal_approx_fast(zinv[:], zrow[:])
                            zib = pat.tile([1, 512], dt.bfloat16,
                                           name=f"zinvb{b}{h}", tag=f"zib{b}{h}")
                            nc.vector.tensor_copy(zib[:], zinv[:])
                            zinvb.append(zib)
                        prev["zinvb"][b] = zinvb

                prev = None
                for qt in range(T // 512):
                    nj = 4 * qt + 4
                    gsl = {b: slice(b * T + 512 * qt, b * T + 512 * qt + 512)
                           for b in range(B)}
                    pc = None
                    ets = {b: [] for b in range(B)}
                    for j in range(nj):
                        for b in range(B):
                            st = b * (T // 128) + j   # global s-tile index
                            et2 = []
                            for h in range(2):
                                hsl = slice(64 * h, 64 * (h + 1))
                                ps = pps_sc.tile([128, 512], dt.float32,
                                                 name="ps", tag="sc")
                                nc.tensor.matmul(
                                    ps[:], kT[hsl, 128 * st:128 * (st + 1)],
                                    qT[hsl, gsl[b]], start=True, stop=True)
                                et = pexp.tile([128, 512], dt.bfloat16,
                                               name=f"et{b}{h}", tag=f"et{b}{h}")
                                if j >= nj - 4:
                                    off = j - (nj - 4)
                                    if off > 0:
                                        nc.gpsimd.memset(et[:, 0:128 * off], 0.0)
                                    nc.scalar.activation(
                                        et[:, 128 * off:512], ps[:, 128 * off:512],
                                        act.Exp, scale=1.0 / float(np.sqrt(H)))
                                    nc.gpsimd.tensor_tensor(
                                        et[:, 128 * off:128 * (off + 1)],
                                        et[:, 128 * off:128 * (off + 1)],
                                        maskd[:], alu.mult)
                                else:
                                    nc.scalar.activation(et[:], ps[:], act.Exp,
                                                         scale=1.0 / float(np.sqrt(H)))
                                et2.append(et)
                            ets[b].append(et2)
                        if j == 0 and prev is not None:
                            emit_tail(prev)
                        if j == 1:
                            if prev is not None:
                                for b in range(B):
                                    emit_norm((prev["pc"][b], prev["zinvb"][b],
                                               prev["gsl"][b], prev["cidx"][b]))
                                prev = None
                            pc = {b: [pps_ctx.tile([65, 512], dt.float32,
                                                   name=f"pc{b}{h}", tag=f"ctx{b}{h}")
                                      for h in range(2)] for b in range(B)}
                        if j > 0:
                            for b in range(B):
                                for h in range(2):
                                    nc.tensor.matmul(
                                        pc[b][h][:], v[:, b * (T // 128) + j - 1, h, :],
                                        ets[b][j - 1][h][:],
                                        start=(j - 1 == 0), stop=False)
                    prev = {"pc": pc,
                            "et_last": {b: ets[b][nj - 1] for b in range(B)},
                            "vidx": {b: b * (T // 128) + nj - 1 for b in range(B)},
                            "gsl": gsl,
                            "cidx": {b: b * (T // 512) + qt for b in range(B)},
                            "zinvb": {}}
                emit_tail(prev)
                for b in range(B):
                    emit_norm((prev["pc"][b], prev["zinvb"][b],
                               prev["gsl"][b], prev["cidx"][b]))
                prev = None

            # ---------------- AllToAll: heads -> tokens ----------------
            nc.gpsimd.collective_compute(
                "AllToAll", alu.bypass,
                replica_groups=[list(range(N_CORES))],
                ins=[cc_in.opt()],
                outs=[cc_out.opt()],
            )

            # ---------------- stage C: Wo + LN2 + FFN ----------------
            with (
                tc.tile_pool(name="postsb", bufs=1) as pq,
                tc.tile_pool(name="wstream", bufs=3) as pw,
                tc.tile_pool(name="evict", bufs=3) as pev,
                tc.tile_pool(name="ln2tmp", bufs=1) as pl2,
                tc.tile_pool(name="ffpsum", bufs=2, space="PSUM") as pps_ff,
                tc.tile_pool(name="cpsum", bufs=1, space="PSUM") as pps_c,
            ):
                ctxF = pq.tile([128, NKC, TS], dt.bfloat16)
                nc.sync.dma_start(ctxF[:],
                                  cc_out[:].rearrange("c p t -> p c t"))

                boc = pq.tile([128, NKC], dt.float32)
                b2c = pq.tile([128, NKC], dt.float32)
                b1c = pq.tile([128, NMF], dt.float32)
                cs1 = pq.tile([1, FF], dt.bfloat16)
                xTs = pq.tile([128, NKC, TS], dt.float32)
                with tc.tile_wait_until(0.12):
                    nc.sync.dma_start(boc[:], p_boc[:])
                    nc.sync.dma_start(b2c[:], p_b2c[:])
                    nc.sync.dma_start(b1c[:], p_b1c[:])
                    nc.sync.dma_start(cs1[:], p_cs1[:])
                    nc.sync.dma_start(xTs[:], p_xTs[:])

                # Wo + residual; LN2 column sums interleaved per block
                r2T = pq.tile([128, NKC, TS], dt.float32)
                r2b = pl2.tile([128, NKC, TS], dt.bfloat16)
                sq = pl2.tile([128, NKC, TS], dt.bfloat16)
                ps1 = pps_c.tile([1, TS], dt.float32, tag="s1")
                ps2 = pps_c.tile([1, TS], dt.float32, tag="s2")
                for mc in range(NKC):
                    wo_blk = pw.tile([128, NKC, 128], dt.bfloat16, tag="wo")
                    with tc.tile_wait_until(0.12):
                        nc.sync.dma_start(wo_blk[:], p_woblk[mc])
                    ps = pps_ff.tile([128, TS], dt.float32, tag="ff")
                    for k in range(NKC):
                        nc.tensor.matmul(ps[:], wo_blk[:, k, :], ctxF[:, k, :],
                                         start=(k == 0), stop=(k == NKC - 1))
                    nc.vector.scalar_tensor_tensor(r2T[:, mc, :], ps[:],
                                                   boc[:, mc:mc + 1],
                                                   xTs[:, mc, :], alu.add, alu.add)
                    nc.scalar.copy(r2b[:, mc, :], r2T[:, mc, :])
                    nc.vector.tensor_tensor(sq[:, mc, :], r2b[:, mc, :],
                                            r2b[:, mc, :], alu.mult)
                    nc.tensor.matmul(ps1[:], isc_col[:], r2b[:, mc, :],
                                     start=(mc == 0), stop=(mc == NKC - 1))
                    nc.tensor.matmul(ps2[:], isc_col[:], sq[:, mc, :],
                                     start=(mc == 0), stop=(mc == NKC - 1))

                # ---- LN2 scalars: -mu row and broadcast 1/(std+eps) ----
                muf = pl2.tile([1, TS], dt.float32)
                nc.vector.tensor_copy(muf[:], ps1[:])
                negmu2 = pl2.tile([1, TS], dt.bfloat16)
                nc.vector.tensor_scalar(negmu2[:], muf[:], -1.0, None, alu.mult)
                varf = pl2.tile([1, TS], dt.float32)
                nc.vector.tensor_tensor(varf[:], muf[:], muf[:], alu.mult)
                nc.vector.tensor_tensor(varf[:], ps2[:], varf[:], alu.subtract)
                stdf2 = pl2.tile([1, TS], dt.float32)
                nc.scalar.activation(stdf2[:], varf[:], act.Sqrt,
                                     scale=float(C) / (C - 1))
                nc.vector.tensor_scalar(stdf2[:], stdf2[:], EPS, None, alu.add)
                inv2 = pl2.tile([1, TS], dt.float32)
                nc.vector.reciprocal_approx_fast(inv2[:], stdf2[:])
                inv2row = pl2.tile([1, TS], dt.bfloat16)
                nc.vector.tensor_copy(inv2row[:], inv2[:])
                pib = pps_c.tile([128, TS], dt.float32, tag="bcast")
                nc.tensor.matmul(pib[:], ones128_row[:], inv2row[:],
                                 start=True, stop=True)
                i2b = pl2.tile([128, TS], dt.bfloat16)
                nc.scalar.copy(i2b[:], pib[:])

                # ---- FFN (LN2 folded into the W1 matmul) ----
                hT = pq.tile([128, NMF, TS], dt.bfloat16)
                for mf in range(NMF):
                    w1_blk = pw.tile([128, NKC, 128], dt.bfloat16, tag="w1")
                    with tc.tile_wait_until(0.15):
                        nc.sync.dma_start(w1_blk[:], p_w1blk[mf])
                    ps = pps_ff.tile([128, TS], dt.float32, tag="ff")
                    for k in range(NKC):
                        nc.tensor.matmul(ps[:], w1_blk[:, k, :], r2b[:, k, :],
                                         start=(k == 0), stop=False)
                    nc.tensor.matmul(ps[:], cs1[0:1, 128 * mf:128 * (mf + 1)],
                                     negmu2[:], start=False, stop=True)
                    tmp = pev.tile([128, TS], dt.bfloat16, tag="htmp")
                    nc.vector.tensor_tensor(tmp[:], ps[:], i2b[:], alu.mult)
                    nc.vector.tensor_scalar(hT[:, mf, :], tmp[:], b1c[:, mf:mf + 1],
                                            0.0, alu.add, alu.max)

                for mc in range(NKC):
                    w2_blk = pw.tile([128, NMF, 128], dt.bfloat16, tag="w2")
                    with tc.tile_wait_until(0.18):
                        nc.sync.dma_start(w2_blk[:], p_w2blk[mc])
                    ps = pps_ff.tile([128, TS], dt.float32, tag="ff")
                    for k in range(NMF):
                        nc.tensor.matmul(ps[:], w2_blk[:, k, :], hT[:, k, :],
                                         start=(k == 0), stop=(k == NMF - 1))
                    ot = pev.tile([128, TS], dt.float32, tag="ot")
                    nc.vector.scalar_tensor_tensor(ot[:], ps[:], b2c[:, mc:mc + 1],
                                                   r2T[:, mc, :], alu.add, alu.add)
                    nc.sync.dma_start(p_out[128 * mc:128 * (mc + 1), :], ot[:])

    nc.compile()
    return nc


def _host_prep(inputs):
    """Fold layernorm affine params into weights; build per-core input maps."""
    x = np.asarray(inputs["x"], np.float32)
    Wq = np.asarray(inputs["Wq"], np.float32)
    Wk = np.asarray(inputs["Wk"], np.float32)
    Wv = np.asarray(inputs["Wv"], np.float32)
    Wo = np.asarray(inputs["Wo"], np.float32)
    bo = np.asarray(inputs["bo"], np.float32)
    W1 = np.asarray(inputs["W1"], np.float32)
    b1 = np.asarray(inputs["b1"], np.float32)
    W2 = np.asarray(inputs["W2"], np.float32)
    b2 = np.asarray(inputs["b2"], np.float32)
    g1 = np.asarray(inputs["g1"], np.float32)
    be1 = np.asarray(inputs["be1"], np.float32)
    g2 = np.asarray(inputs["g2"], np.float32)
    be2 = np.asarray(inputs["be2"], np.float32)

    xf = x.reshape(TT, C)                      # both batches stacked
    xT = np.ascontiguousarray(xf.T)            # [C, TT]
    # chunk-major, partition-contiguous: [NCH, 128, NKC, 512]
    xT_blk = np.ascontiguousarray(
        xT.reshape(NKC, 128, NCH, 512).transpose(2, 1, 0, 3))

    def fold_qkv(W):
        Weff = g1[:, None] * W                  # [NH, C, H] with g1 on C
        Weff = np.ascontiguousarray(np.transpose(Weff, (1, 0, 2)))  # [C, NH, H]
        bias = np.einsum("c,hck->hk", be1, W)   # [NH, H]
        colsum = Weff.sum(axis=0)               # [NH, H]
        return Weff, bias, colsum

    Wq_e, bq, csq = fold_qkv(Wq)
    Wk_e, bk, csk = fold_qkv(Wk)
    Wv_e, bv, csv = fold_qkv(Wv)

    woT = np.ascontiguousarray(Wo.T)            # [NH*H, C]
    w1T = np.ascontiguousarray(g2[:, None] * W1.T)   # [C, FF]
    b1_eff = b1 + be2 @ W1.T                         # [FF]
    cs1 = w1T.sum(axis=0)                            # [FF]
    w2T = np.ascontiguousarray(W2.T)            # [FF, C]

    # blocked weights [mblk, 128, K/128, 128], contiguous per partition
    woblk = np.ascontiguousarray(
        woT.reshape(NKC, 128, NKC, 128).transpose(2, 1, 0, 3))
    w1blk = np.ascontiguousarray(
        w1T.reshape(NKC, 128, NMF, 128).transpose(2, 1, 0, 3))
    w2blk = np.ascontiguousarray(
        w2T.reshape(NMF, 128, NKC, 128).transpose(2, 1, 0, 3))

    tq = np.arange(128)[None, :]
    s = np.arange(128)[:, None]
    maskd = (s <= tq).astype(BF16)

    x_bf = xf.astype(BF16)
    shared = {
        "p_xT": xT_blk.astype(BF16),
        "p_woblk": woblk.astype(BF16),
        "p_boc": np.ascontiguousarray(
            bo.reshape(NKC, 128).T).astype(np.float32),
        "p_w1blk": w1blk.astype(BF16),
        "p_b1c": np.ascontiguousarray(
            b1_eff.reshape(NMF, 128).T).astype(np.float32),
        "p_cs1": cs1[None, :].astype(BF16),
        "p_w2blk": w2blk.astype(BF16),
        "p_b2c": np.ascontiguousarray(
            b2.reshape(NKC, 128).T).astype(np.float32),
        "p_maskd": maskd,
        "p_ident": np.eye(128, dtype=np.float32).astype(BF16),
    }

    in_maps = []
    for r in range(N_CORES):
        h0 = HPC * r
        hs = slice(h0, h0 + HPC)
        b_r, s_r = divmod(r, N_CORES // B)
        tok = slice(s_r * TS, (s_r + 1) * TS)
        xTs = np.ascontiguousarray(
            x[b_r].T[:, tok].reshape(NKC, 128, TS).transpose(1, 0, 2))
        m = dict(shared)
        m["p_x"] = x_bf[r * (TT // N_CORES):(r + 1) * (TT // N_CORES), :]
        m["p_xTs"] = xTs.astype(np.float32)
        m["p_wq"] = np.ascontiguousarray(
            Wq_e[:, hs, :].reshape(NKC, 128, HD2).transpose(1, 0, 2)).astype(BF16)
        m["p_wk"] = np.ascontiguousarray(
            Wk_e[:, hs, :].reshape(NKC, 128, HD2).transpose(1, 0, 2)).astype(BF16)
        m["p_wv"] = np.ascontiguousarray(
            Wv_e[:, hs, :].reshape(NKC, 128, HD2).transpose(1, 0, 2)).astype(BF16)
        m["p_cq"] = np.stack([csq[hs].reshape(HD2),
                              bq[hs].reshape(HD2)]).astype(BF16)
        m["p_ck"] = np.stack([csk[hs].reshape(HD2),
                              bk[hs].reshape(HD2)]).astype(BF16)
        m["p_cv"] = np.stack([csv[hs].reshape(HD2),
                              bv[hs].reshape(HD2)]).astype(BF16)
        in_maps.append(m)
    return in_maps


def kernel(**inputs) -> np.ndarray:
    from concourse.bass_utils import run_bass_kernel_spmd

    if "nc" not in _BUILT:
        _BUILT["nc"] = _build()
    nc = _BUILT["nc"]

    in_maps = _host_prep(inputs)
    res = run_bass_kernel_spmd(nc, in_maps, core_ids=list(range(N_CORES)))

    out = np.empty((B, T, C), np.float32)
    for r in range(N_CORES):
        b_r, s_r = divmod(r, N_CORES // B)
        out[b_r, s_r * TS:(s_r + 1) * TS, :] = res.results[r]["p_out"].T
    return out
